# revision 5
# baseline (speedup 1.0000x reference)
"""Trainium2 Bass kernel for nn_MetricConv (GNN message passing).

Math (see reference):
  nc = [stage_start | context | stage_end]            [N, 256]
  cl = nc @ W_l + b_l ; cr = nc @ W_r + b_r           [N, 256]
  per edge (src j -> dst i):  ctx = selu(cr[dst] + cl[src])
  alpha = ctx @ att ; mask = alpha != 0
  softmax over edges grouped by dst (max-subtraction skipped: |alpha| is
  small for this model family, exp() cannot overflow, and the max factor
  cancels exactly in ex/s; verified numerically in test.py)
  h = selu([ctx | sm[src]] @ W1 + b1) ; f = selu(h @ W2 + b2)
  out[n] = (sum_e ex_e * f_e) / (sum_e ex_e + 1e-16) over masked edges
  rows with no contribution -> stage_metrics[n], else sigmoid(out + bias)

Distribution: edges are sorted by dst on the host and partitioned by dst
range across 8 cores (no collectives needed).  Each core aggregates its
own 12500-node slice.  Per 128-node window the scatter-add is a one-hot
matmul accumulated in PSUM; per-window tile counts are equalized across
cores (max over cores) so a single SPMD program serves all 8 cores.

selu(x) = lam*relu(x) + lam*alph*(min(exp(x),1) - 1)   (exact identity)
"""
import math
import numpy as np

import concourse.bacc as bacc
import concourse.tile as tile
import concourse.bass as bass
from concourse import mybir
from concourse import bass_utils
from concourse.masks import make_identity

F32 = mybir.dt.float32
I32 = mybir.dt.int32
AF = mybir.ActivationFunctionType
ALU = mybir.AluOpType
AX = mybir.AxisListType

LAM = 1.0507009873554804934193349852946
ALPH = 1.6732632423543772848170429916717
LA = LAM * ALPH
P = 128

# ---------------------------------------------------------------- config ----


class Cfg:
    def __init__(self, n_nodes, n_edges, ncores):
        self.N = n_nodes
        self.E = n_edges
        self.NCORES = ncores
        self.DS, self.DC, self.DM = 16, 224, 128
        self.CC = 2 * self.DS + self.DC          # 256
        self.H = (self.CC + self.DM) // 2        # 192
        self.OUT = self.DM                       # 128
        self.CORE_NODES = n_nodes // ncores
        self.WINDOWS = math.ceil(self.CORE_NODES / P)
        self.CORE_PAD = self.WINDOWS * P
        self.NPAD = math.ceil((n_nodes + 1) / P) * P
        self.DUMMY = n_nodes                     # index of the all-zero row


# ------------------------------------------------------------- host prep ----


def host_prepare(cfg, edge_index, stage_start, stage_end, context,
                 stage_metrics, W_l, b_l, W_r, b_r, att, W1, b1, W2, b2, bias):
    """All numpy staging: concat, sort, partition, frame layout, weight
    reshaping.  Returns (struct, in_maps)."""
    N, E, NC = cfg.N, cfg.E, cfg.NCORES
    CC, DM, H, OUT = cfg.CC, cfg.DM, cfg.H, cfg.OUT

    ncfeat = np.zeros((cfg.NPAD, CC), np.float32)
    ncfeat[:N, :cfg.DS] = stage_start
    ncfeat[:N, cfg.DS:cfg.DS + cfg.DC] = context
    ncfeat[:N, cfg.DS + cfg.DC:] = stage_end

    sm_tab = np.zeros((cfg.NPAD, DM), np.float32)
    sm_tab[:N] = stage_metrics

    src = np.asarray(edge_index[0], np.int64)
    dst = np.asarray(edge_index[1], np.int64)
    order = np.argsort(dst, kind="stable")
    src_s = src[order].astype(np.int32)
    dst_s = dst[order].astype(np.int32)

    # per (core, window) edge counts -> shared per-window tile counts
    core_starts = np.searchsorted(dst_s, np.arange(NC) * cfg.CORE_NODES)
    core_ends = np.searchsorted(dst_s, (np.arange(NC) + 1) * cfg.CORE_NODES)
    counts = np.zeros((NC, cfg.WINDOWS), np.int64)
    win_edges = {}
    for c in range(NC):
        s0, s1 = core_starts[c], core_ends[c]
        dl = dst_s[s0:s1] - c * cfg.CORE_NODES
        wb = np.searchsorted(dl, np.arange(cfg.WINDOWS + 1) * P)
        for w in range(cfg.WINDOWS):
            counts[c, w] = wb[w + 1] - wb[w]
            win_edges[(c, w)] = (s0 + wb[w], s0 + wb[w + 1])
    T_w = np.maximum(1, np.ceil(counts.max(axis=0) / P).astype(np.int64))
    Ttot = int(T_w.sum())

    # frame arrays, padded; layout [P, Ttot] partition-major (slot p of tile t
    # at [p, t])
    srcg = np.full((NC, Ttot * P), cfg.DUMMY, np.int32)
    crloc = np.full((NC, Ttot * P), cfg.CORE_PAD - 1, np.int32)
    dshift = np.full((NC, Ttot * P), 1.0e6, np.float32)
    tile_base = np.concatenate([[0], np.cumsum(T_w)])
    for c in range(NC):
        for w in range(cfg.WINDOWS):
            e0, e1 = win_edges[(c, w)]
            k = e1 - e0
            off = tile_base[w] * P
            srcg[c, off:off + k] = src_s[e0:e1]
            crloc[c, off:off + k] = dst_s[e0:e1] - c * cfg.CORE_NODES
            dshift[c, off:off + k] = (dst_s[e0:e1] - c * cfg.CORE_NODES
                                      - w * P).astype(np.float32)

    def pm(a, dt):  # [NC, Ttot*P] -> [NC, P, Ttot] partition-major
        return np.ascontiguousarray(
            a.reshape(NC, Ttot, P).transpose(0, 2, 1)).astype(dt)

    srcg_pm, crloc_pm, dsh_pm = (pm(srcg, np.int32), pm(crloc, np.int32),
                                 pm(dshift, np.float32))

    W_l = np.asarray(W_l, np.float32)
    W_r = np.asarray(W_r, np.float32)
    W1 = np.asarray(W1, np.float32)
    W2 = np.asarray(W2, np.float32)
    w2b = np.concatenate([W2[P:H], np.asarray(b2, np.float32)[None, :]], 0)

    rep = lambda v, n: np.repeat(np.asarray(v, np.float32)[None, :], n, 0)
    col = lambda v: np.ascontiguousarray(np.asarray(v, np.float32)[:, None])

    common = {
        "wl0": W_l[0:P], "wl1": W_l[P:CC], "wr0": W_r[0:P], "wr1": W_r[P:CC],
        "w1k0": W1[0:P], "w1k1": W1[P:2 * P], "w1k2": W1[2 * P:CC + DM],
        "w2a": W2[0:P], "w2b": w2b,
        "att_rep": rep(att, P), "blrep": rep(b_l, P), "brrep": rep(b_r, P),
        "biasrep": rep(bias, P),
        "b1a": col(b1[0:P]), "b1b": col(b1[P:H]),
        "b1la": col(b1[0:P] * LAM), "b1lb": col(b1[P:H] * LAM),
        "ncfeat": ncfeat, "sm_tab": sm_tab,
    }
    in_maps = []
    for c in range(NC):
        m = dict(common)
        m["ncfeat_own"] = np.ascontiguousarray(
            ncfeat[c * cfg.CORE_NODES:c * cfg.CORE_NODES + cfg.CORE_PAD])
        m["sm_own"] = np.ascontiguousarray(
            sm_tab[c * cfg.CORE_NODES:c * cfg.CORE_NODES + cfg.CORE_PAD])
        m["srcg"] = srcg_pm[c]
        m["crloc"] = crloc_pm[c]
        m["dsh"] = dsh_pm[c]
        in_maps.append(m)

    struct = {"T_w": tuple(int(t) for t in T_w), "Ttot": Ttot}
    return struct, in_maps


# --------------------------------------------------------- device program ---


def build_program(cfg, struct):
    T_w, Ttot = struct["T_w"], struct["Ttot"]
    CC, DM, H, OUT = cfg.CC, cfg.DM, cfg.H, cfg.OUT
    NPAD, CPAD, WINDOWS = cfg.NPAD, cfg.CORE_PAD, cfg.WINDOWS
    NTILES = NPAD // P

    nc = bacc.Bacc("TRN2", target_bir_lowering=False, debug=False,
                   enable_asserts=False, num_devices=cfg.NCORES)
    din = lambda n, s: nc.dram_tensor(n, s, F32, kind="ExternalInput").ap()
    dini = lambda n, s: nc.dram_tensor(n, s, I32, kind="ExternalInput").ap()

    ncfeat = din("ncfeat", [NPAD, CC])
    ncfeat_own = din("ncfeat_own", [CPAD, CC])
    sm_tab = din("sm_tab", [NPAD, DM])
    sm_own = din("sm_own", [CPAD, DM])
    wl0, wl1 = din("wl0", [P, CC]), din("wl1", [P, CC])
    wr0, wr1 = din("wr0", [P, CC]), din("wr1", [P, CC])
    w1k0, w1k1, w1k2 = (din("w1k0", [P, H]), din("w1k1", [P, H]),
                        din("w1k2", [P, H]))
    w2a, w2b = din("w2a", [P, OUT]), din("w2b", [H - P + 1, OUT])
    att_rep = din("att_rep", [P, CC])
    blrep, brrep = din("blrep", [P, CC]), din("brrep", [P, CC])
    biasrep = din("biasrep", [P, OUT])
    b1a, b1b = din("b1a", [P, 1]), din("b1b", [H - P, 1])
    b1la, b1lb = din("b1la", [P, 1]), din("b1lb", [H - P, 1])
    srcg_d = dini("srcg", [P, Ttot])
    crloc_d = dini("crloc", [P, Ttot])
    dsh_d = din("dsh", [P, Ttot])
    out_tab = nc.dram_tensor("out_tab", [CPAD, OUT], F32,
                             kind="ExternalOutput").ap()

    with tile.TileContext(nc) as tc:
        import contextlib
        with contextlib.ExitStack() as top:
            cn = top.enter_context(tc.tile_pool(name="cn", bufs=1))
            dr = top.enter_context(tc.tile_pool(name="dr", bufs=1,
                                                space="DRAM"))
            cl_tab = dr.tile([NPAD, CC], F32)
            cr_tab = dr.tile([CPAD, CC], F32)

            ident = cn.tile([P, P], F32)
            make_identity(nc, ident[:])
            iota_i = cn.tile([P, P], I32)
            nc.gpsimd.iota(iota_i[:], pattern=[[1, P]], base=0,
                           channel_multiplier=0)
            iota_rep = cn.tile([P, P], F32)
            nc.vector.tensor_copy(iota_rep[:], iota_i[:])
            ones128 = cn.tile([P, P], F32)
            nc.vector.memset(ones128[:], 1.0)

            # resident weights / index arrays
            def load(ap, shape, dt=F32):
                t = cn.tile(shape, dt, tag=f"cn_{ap.tensor.name}")
                nc.sync.dma_start(t[:], ap[:])
                return t
            WL0, WL1 = load(wl0, [P, CC]), load(wl1, [P, CC])
            WR0, WR1 = load(wr0, [P, CC]), load(wr1, [P, CC])
            W1K = [load(w1k0, [P, H]), load(w1k1, [P, H]), load(w1k2, [P, H])]
            W2A, W2B = load(w2a, [P, OUT]), load(w2b, [H - P + 1, OUT])
            ATT = load(att_rep, [P, CC])
            BL, BR = load(blrep, [P, CC]), load(brrep, [P, CC])
            BIAS = load(biasrep, [P, OUT])
            B1A, B1B = load(b1a, [P, 1]), load(b1b, [H - P, 1])
            B1LA, B1LB = load(b1la, [P, 1]), load(b1lb, [H - P, 1])
            SRC = load(srcg_d, [P, Ttot], I32)
            CRL = load(crloc_d, [P, Ttot], I32)
            DSH = load(dsh_d, [P, Ttot])

            # ---------------- phase N: node transform -> cl/cr tables ------
            with tc.tile_pool(name="nsb", bufs=3) as nsb, \
                 tc.tile_pool(name="nps", bufs=2, space="PSUM") as nps:

                def node_tile(src_ap, row, Ws, brep, dst_tab):
                    nf = nsb.tile([P, CC], F32, tag="nf")
                    nc.sync.dma_start(nf[:], src_ap[row:row + P, :])
                    ntp = nps.tile([P, CC], F32, space="PSUM", tag="ntp")
                    nc.tensor.transpose(out=ntp[:, 0:P], in_=nf[:, 0:P],
                                        identity=ident[:])
                    nc.tensor.transpose(out=ntp[:, P:CC], in_=nf[:, P:CC],
                                        identity=ident[:])
                    nfT = nsb.tile([P, CC], F32, tag="nfT")
                    nc.scalar.copy(nfT[:, 0:P], ntp[:, 0:P])
                    nc.scalar.copy(nfT[:, P:CC], ntp[:, P:CC])
                    ps = nps.tile([P, CC], F32, space="PSUM", tag="clps")
                    nc.tensor.matmul(out=ps[:], lhsT=nfT[:, 0:P], rhs=Ws[0][:],
                                     start=True, stop=False)
                    nc.tensor.matmul(out=ps[:], lhsT=nfT[:, P:CC], rhs=Ws[1][:],
                                     start=False, stop=True)
                    v = nsb.tile([P, CC], F32, tag="clv")
                    nc.vector.tensor_tensor(out=v[:], in0=ps[:], in1=brep[:],
                                            op=ALU.add)
                    nc.sync.dma_start(dst_tab[row:row + P, :], v[:])

                for i in range(NTILES):
                    node_tile(ncfeat, i * P, (WL0, WL1), BL, cl_tab[:])
                for i in range(CPAD // P):
                    node_tile(ncfeat_own, i * P, (WR0, WR1), BR, cr_tab[:])

            # ---------------- phase E: edges ------------------------------
            with tc.tile_pool(name="esb", bufs=3) as esb, \
                 tc.tile_pool(name="fsb", bufs=2) as fsb, \
                 tc.tile_pool(name="eps", bufs=2, space="PSUM") as eps, \
                 tc.tile_pool(name="ups", bufs=2, space="PSUM") as ups:

                k = 0
                for w in range(WINDOWS):
                    U = ups.tile([P, OUT + 1], F32, space="PSUM", tag="U")
                    for t in range(T_w[w]):
                        first, last = t == 0, t == T_w[w] - 1
                        cj = esb.tile([P, CC], F32, tag="cj")
                        nc.gpsimd.indirect_dma_start(
                            out=cj[:], out_offset=None, in_=cl_tab[:],
                            in_offset=bass.IndirectOffsetOnAxis(
                                ap=SRC[:, k:k + 1], axis=0))
                        ci = esb.tile([P, CC], F32, tag="ci")
                        nc.gpsimd.indirect_dma_start(
                            out=ci[:], out_offset=None, in_=cr_tab[:],
                            in_offset=bass.IndirectOffsetOnAxis(
                                ap=CRL[:, k:k + 1], axis=0))
                        mjx = esb.tile([P, DM], F32, tag="mjx")
                        nc.gpsimd.indirect_dma_start(
                            out=mjx[:], out_offset=None, in_=sm_tab[:],
                            in_offset=bass.IndirectOffsetOnAxis(
                                ap=SRC[:, k:k + 1], axis=0))

                        x = esb.tile([P, CC], F32, tag="x")
                        nc.vector.tensor_tensor(out=x[:], in0=ci[:], in1=cj[:],
                                                op=ALU.add)
                        ex_ = esb.tile([P, CC], F32, tag="ex_")
                        nc.scalar.activation(ex_[:], x[:], AF.Exp)
                        rx = esb.tile([P, CC], F32, tag="rx")
                        nc.scalar.activation(rx[:], x[:], AF.Relu, scale=LAM)
                        t1 = esb.tile([P, CC], F32, tag="t1")
                        nc.vector.tensor_scalar(t1[:], ex_[:], 1.0, LA,
                                                ALU.min, ALU.mult)
                        ctx = esb.tile([P, CC], F32, tag="ctx")
                        nc.vector.scalar_tensor_tensor(ctx[:], t1[:], LA,
                                                       rx[:], ALU.subtract,
                                                       ALU.add)
                        am = esb.tile([P, CC], F32, tag="am")
                        nc.vector.tensor_tensor(out=am[:], in0=ctx[:],
                                                in1=ATT[:], op=ALU.mult)
                        alpha = esb.tile([P, 1], F32, tag="alpha")
                        nc.vector.tensor_reduce(out=alpha[:], in_=am[:],
                                                axis=AX.X, op=ALU.add)
                        ea = esb.tile([P, 1], F32, tag="ea")
                        nc.scalar.activation(ea[:], alpha[:], AF.Exp)
                        msk = esb.tile([P, 1], F32, tag="msk")
                        nc.vector.tensor_scalar(msk[:], alpha[:], 0.0, None,
                                                ALU.not_equal)
                        exv = esb.tile([P, 1], F32, tag="exv")
                        nc.vector.tensor_tensor(out=exv[:], in0=ea[:],
                                                in1=msk[:], op=ALU.mult)
                        Sp = esb.tile([P, P], F32, tag="Sp")
                        nc.vector.tensor_scalar(Sp[:], iota_rep[:],
                                                DSH[:, k:k + 1], exv[:, :1],
                                                ALU.is_equal, ALU.mult)

                        xt_ps = eps.tile([P, CC + DM], F32, space="PSUM",
                                         tag="xt_ps")
                        nc.tensor.transpose(out=xt_ps[:, 0:P],
                                            in_=ctx[:, 0:P], identity=ident[:])
                        nc.tensor.transpose(out=xt_ps[:, P:CC],
                                            in_=ctx[:, P:CC], identity=ident[:])
                        nc.tensor.transpose(out=xt_ps[:, CC:CC + DM],
                                            in_=mjx[:], identity=ident[:])
                        xt = esb.tile([P, CC + DM], F32, tag="xt")
                        nc.scalar.copy(xt[:, 0:P], xt_ps[:, 0:P])
                        nc.scalar.copy(xt[:, P:CC], xt_ps[:, P:CC])
                        nc.scalar.copy(xt[:, CC:CC + DM], xt_ps[:, CC:CC + DM])

                        h_ps = eps.tile([P, 2 * P], F32, space="PSUM",
                                        tag="h_ps")
                        for kk in range(3):
                            nc.tensor.matmul(
                                out=h_ps[:, 0:P], lhsT=W1K[kk][:, 0:P],
                                rhs=xt[:, kk * P:(kk + 1) * P],
                                start=(kk == 0), stop=(kk == 2))
                        for kk in range(3):
                            nc.tensor.matmul(
                                out=h_ps[0:H - P, P:2 * P],
                                lhsT=W1K[kk][:, P:H],
                                rhs=xt[:, kk * P:(kk + 1) * P],
                                start=(kk == 0), stop=(kk == 2))

                        hA = fsb.tile([P, P], F32, tag="hA")
                        hB = fsb.tile([H - P + 1, P], F32, tag="hB")
                        for (sl, co, bb, bl, ht, hsl) in (
                                (slice(0, P), slice(0, P), B1A, B1LA,
                                 hA, slice(0, P)),
                                (slice(0, H - P), slice(P, 2 * P), B1B, B1LB,
                                 hB, slice(0, H - P))):
                            eh = fsb.tile([P, P], F32, tag=f"eh{co.start}")
                            nc.scalar.activation(eh[sl, :], h_ps[sl, co],
                                                 AF.Exp, bias=bb[:])
                            rh = fsb.tile([P, P], F32, tag=f"rh{co.start}")
                            nc.scalar.activation(rh[sl, :], h_ps[sl, co],
                                                 AF.Relu, bias=bl[:],
                                                 scale=LAM)
                            t1h = fsb.tile([P, P], F32, tag=f"t1h{co.start}")
                            nc.vector.tensor_scalar(t1h[sl, :], eh[sl, :], 1.0,
                                                    LA, ALU.min, ALU.mult)
                            nc.vector.scalar_tensor_tensor(
                                ht[hsl, :], t1h[sl, :], LA, rh[sl, :],
                                ALU.subtract, ALU.add)
                        nc.gpsimd.memset(hB[H - P:H - P + 1, :], 1.0)

                        f_ps = eps.tile([P, OUT], F32, space="PSUM",
                                        tag="f_ps")
                        nc.tensor.matmul(out=f_ps[:], lhsT=hA[:], rhs=W2A[:],
                                         start=True, stop=False)
                        nc.tensor.matmul(out=f_ps[:], lhsT=hB[:], rhs=W2B[:],
                                         start=False, stop=True)
                        ef = fsb.tile([P, OUT], F32, tag="ef")
                        nc.scalar.activation(ef[:], f_ps[:], AF.Exp)
                        rf = fsb.tile([P, OUT], F32, tag="rf")
                        nc.scalar.activation(rf[:], f_ps[:], AF.Relu,
                                             scale=LAM)
                        t1f = fsb.tile([P, OUT], F32, tag="t1f")
                        nc.vector.tensor_scalar(t1f[:], ef[:], 1.0, LA,
                                                ALU.min, ALU.mult)
                        fsb_t = fsb.tile([P, OUT + 1], F32, tag="fsb_t")
                        nc.vector.scalar_tensor_tensor(
                            fsb_t[:, 0:OUT], t1f[:], LA, rf[:],
                            ALU.subtract, ALU.add)
                        nc.gpsimd.memset(fsb_t[:, OUT:OUT + 1], 1.0)

                        nc.tensor.matmul(out=U[:], lhsT=Sp[:], rhs=fsb_t[:],
                                         start=first, stop=last,
                                         skip_group_check=True)
                        k += 1

                    # -------- finalize window w --------
                    se = esb.tile([P, 1], F32, tag="se")
                    nc.vector.tensor_scalar(se[:], U[:, OUT:OUT + 1], 1e-16,
                                            None, ALU.add)
                    rec = esb.tile([P, 1], F32, tag="rec")
                    nc.vector.reciprocal(rec[:], se[:])
                    outn = esb.tile([P, OUT], F32, tag="outn")
                    nc.vector.tensor_scalar(outn[:], U[:, 0:OUT], rec[:, :1],
                                            None, ALU.mult)
                    rabs = esb.tile([P, 1], F32, tag="rabs")
                    nc.vector.tensor_reduce(out=rabs[:], in_=outn[:], axis=AX.X,
                                            op=ALU.max,
                                            apply_absolute_value=True)
                    flag = esb.tile([P, 1], F32, tag="flag")
                    nc.vector.tensor_scalar(flag[:], rabs[:], 0.0, None,
                                            ALU.is_equal)
                    flagrep = esb.tile([P, OUT], I32, tag="flagrep")
                    nc.vector.tensor_scalar(flagrep[:], ones128[:, 0:OUT],
                                            flag[:, :1], None, ALU.mult)
                    sigin = esb.tile([P, OUT], F32, tag="sigin")
                    nc.vector.tensor_tensor(out=sigin[:], in0=outn[:],
                                            in1=BIAS[:], op=ALU.add)
                    sig = esb.tile([P, OUT], F32, tag="sig")
                    nc.scalar.activation(sig[:], sigin[:], AF.Sigmoid)
                    smw = esb.tile([P, DM], F32, tag="smw")
                    nc.sync.dma_start(smw[:], sm_own[w * P:(w + 1) * P, :])
                    resv = esb.tile([P, OUT], F32, tag="resv")
                    nc.vector.tensor_copy(resv[:], sig[:])
                    nc.vector.copy_predicated(resv[:], flagrep[:], smw[:])
                    nc.sync.dma_start(out_tab[w * P:(w + 1) * P, :], resv[:])

    nc.compile()
    return nc


# ------------------------------------------------------------------ entry ---

_CACHE = {}
LAST_EXEC_NS = None
LAST_RUN_WALL_NS = None


def _get_program(cfg, struct):
    key = (cfg.N, cfg.E, cfg.NCORES, struct["T_w"])
    if key not in _CACHE:
        _CACHE[key] = build_program(cfg, struct)
    return _CACHE[key]


def run(cfg, **inputs):
    global LAST_EXEC_NS, LAST_RUN_WALL_NS
    struct, in_maps = host_prepare(cfg, **inputs)
    nc = _get_program(cfg, struct)
    import time as _time
    _t0 = _time.time()
    res = bass_utils.run_bass_kernel_spmd(
        nc, in_maps, core_ids=list(range(cfg.NCORES)))
    LAST_RUN_WALL_NS = int((_time.time() - _t0) * 1e9)
    LAST_EXEC_NS = res.exec_time_ns
    out = np.concatenate(
        [res.results[c]["out_tab"][:cfg.CORE_NODES]
         for c in range(cfg.NCORES)], axis=0)
    return out.astype(np.float32)


def kernel(**inputs):
    cfg = Cfg(100000, 1000000, 8)
    args = {k: np.asarray(v) for k, v in inputs.items()}
    return run(cfg, **args)
